# revision 15
# baseline (speedup 1.0000x reference)
"""Trainium2 Bass kernel for nn_CrossNetwork: 4-layer cross-network.

Reference semantics (per row b of x [B, D], D=512, L=4 layers):
    x_list = [x]
    for i in range(L):
        h = x_list[-1]
        for p in x_list[:-1]:          # sequential dot-product residuals
            s = <h_cur, p>             # scalar per row (h_cur updated each step)
            h_cur = h_cur + s * ones
        y = h_cur @ W[i].T + b[i]
        x_list.append(y)
    out = concat(x_list[1:])           # [B, L*D]

Algebraic restructure (exact): with D_j = <h, p_j> (h unmodified) and
sig_j = rowsum(p_j), the recurrence s'_j = D_j + S_{<j}*sig_j, S = sum s'_j
gives x_fin = h + S.  Then
    y = x_fin @ W_i^T + b = (h @ W_i^T) + S * r_i + b,   r_i = rowsum(W_i)
so the GEMM runs on h = y_{i-1} directly and the cross term is a rank-1
update applied during PSUM evacuation:
    y = (r_bcast * S_col) + z        (z = h @ W_i^T + b via PE accumulation)

Scheduling: engine queues execute in program order, so work is emitted as a
WAVEFRONT: tile t's layer i lands at slot s = t + 4*i; each slot mixes
tiles at different layers, smoothing the layer-dependent dot load.  All 16
tiles' activations stay SBUF-resident.  PE warm-up dummies + front-loaded
x/W0 DMA kicks (spread across engine queues) avoid the cold-clock start.

The per-row scalar recurrence is BATCHED over groups of 4 tiles: dot/sigma
accumulators land in per-group [128, 4] column blocks, and the recurrence
runs as a handful of tiny [128, 4] tensor_tensor ops on DVE instead of
16x as many per-tile column ops.

Engine mapping per (tile, layer):
  PE : 4 transposes of h chunks, 4 GEMM matmuls (N=512, PSUM-accumulate),
       1 bias matmul (K=1, ones x b_row)
  ACT: PSUM->SBUF bf16 copy of transposed chunks; layer-0 evacuation
       (sigma0 via accum_out); reduce half of GPSIMD-multiplied dots
  DVE: most dots as one fused scalar_tensor_tensor (accum_out -> D col);
       rank-1 fused evacuation stt (sigma1 via accum_out); batched
       recurrence [128,4] ops
  GPS: multiply half of two layer-3 dots per tile; x-tile DMA kicks

Everything bf16 (x, W, activations, output -> 13 MiB DMA/core); output is
written bf16, one 512 KB DMA per tile, upcast on host.  Rel err ~7e-3.
Sharding: batch split across 8 NeuronCores (data parallel, SPMD).
"""

import numpy as np

NUM_LAYERS = 4
D = 512
B = 16384
N_CORES = 8
ROWS_PER_CORE = B // N_CORES          # 2048
NTILES = ROWS_PER_CORE // 128         # 16
NCH = D // 128                        # 4 contraction chunks
WAVE = 5                              # slot stagger between layers
GRP = 4                               # tiles per recurrence batch group

_CACHE = {}


def _build_nc(ntiles=NTILES):
    import concourse.tile as tile
    from concourse import bacc, mybir
    from concourse.masks import make_identity

    F32 = mybir.dt.float32
    BF16 = mybir.dt.bfloat16
    AF = mybir.ActivationFunctionType
    MUL = mybir.AluOpType.mult
    ADD = mybir.AluOpType.add

    rows = ntiles * 128
    ngrp = ntiles // GRP

    nc = bacc.Bacc("TRN2", target_bir_lowering=False, debug=False)

    X = nc.dram_tensor("x", [rows, D], BF16, kind="ExternalInput")
    WT = nc.dram_tensor("wt", [NUM_LAYERS, D, D], BF16, kind="ExternalInput")
    BIAS = nc.dram_tensor("bias", [NUM_LAYERS, D], BF16, kind="ExternalInput")
    RB = nc.dram_tensor("rb", [NUM_LAYERS, 128, D], F32, kind="ExternalInput")
    OUT = nc.dram_tensor("out", [rows, NUM_LAYERS * D], BF16,
                         kind="ExternalOutput")

    with tile.TileContext(nc) as tc:
        with (
            tc.tile_pool(name="consts", bufs=1) as consts,
            tc.tile_pool(name="xs", bufs=1) as xs,
            tc.tile_pool(name="ys", bufs=1) as ys,
            tc.tile_pool(name="scals", bufs=1) as scals,
            tc.tile_pool(name="xTs", bufs=6) as xTs,
            tc.tile_pool(name="prods", bufs=4) as prods,
            tc.tile_pool(name="zpsum", bufs=5, space="PSUM") as zpsum,
            tc.tile_pool(name="tpsum", bufs=3, space="PSUM") as tpsum,
        ):
            # ---- identity + PE warm-up (runs while DMAs land) ----
            ones_f32 = consts.tile([1, 128], F32)
            nc.vector.memset(ones_f32[:], 1.0)
            ones_col = consts.tile([1, 128], BF16)
            nc.vector.tensor_copy(ones_col[:], ones_f32[:])
            ident = consts.tile([128, 128], BF16)
            make_identity(nc, ident[:])
            warm = tpsum.tile([128, NCH, 128], BF16, tag="tr", name="warm")
            for _ in range(60):
                nc.tensor.transpose(warm[:, 0, :], ident[:], ident[:])

            # ---- inputs: x tiles + layer-0 weights first, kicks spread ----
            x_dram = X.rearrange("(t p) d -> t p d", p=128)
            out_dram = OUT.rearrange("(t p) d -> t p d", p=128)
            wt_dram = WT.rearrange("l (c p) e -> l c p e", p=128)

            wt_sb = consts.tile([128, NUM_LAYERS, NCH, D], BF16)
            bias_sb = consts.tile([1, NUM_LAYERS, D], BF16)
            rb_sb = consts.tile([128, NUM_LAYERS, D], F32)

            x_tiles, y_tiles = [], []
            for t in range(ntiles):
                x_t = xs.tile([128, D], BF16, tag=f"x{t}", name=f"x{t}")
                x_tiles.append(x_t)
                y_tiles.append(ys.tile([128, NUM_LAYERS, D], BF16,
                                       tag=f"y{t}", name=f"y{t}"))
            gscal = [scals.tile([128, 64], F32, tag=f"g{g}", name=f"g{g}")
                     for g in range(ngrp)]

            # layer-0 critical loads first; x from GPSIMD queue, W from SP,
            # bias/rb from DVE queue so the kick costs parallelize.
            for t in range(4):
                nc.gpsimd.dma_start(x_tiles[t][:], x_dram[t, :, :])
            for c in range(NCH):
                nc.sync.dma_start(wt_sb[:, 0, c, :], wt_dram[0, c, :, :])
            nc.sync.dma_start(bias_sb[0:1, 0, :], BIAS[0:1, :])
            for t in range(4, ntiles):
                nc.gpsimd.dma_start(x_tiles[t][:], x_dram[t, :, :])
            for i in range(1, NUM_LAYERS):
                for c in range(NCH):
                    nc.sync.dma_start(wt_sb[:, i, c, :], wt_dram[i, c, :, :])
                nc.scalar.dma_start(bias_sb[0:1, i, :], BIAS[i:i + 1, :])
                nc.scalar.dma_start(rb_sb[:, i, :], RB[i, :, :])

            acts = [[x_tiles[t]] for t in range(ntiles)]

            # gscal column layout (all [128, GRP] blocks, k = t % GRP):
            C_D = {(1, 0): 0, (2, 0): 4, (2, 1): 8,
                   (3, 0): 12, (3, 1): 16, (3, 2): 20}   # D_j at layer i
            C_SIG0, C_SIG1 = 24, 28
            C_S2, C_S3 = 32, 36
            C_T1, C_T2 = 40, 44

            def emit_dots(t, i):
                """Dot products for (tile t, layer i) -> accumulator cols."""
                g, k = t // GRP, t % GRP
                h = acts[t][-1]
                for j, p in enumerate(acts[t][:-1]):
                    Dj = gscal[g][:, C_D[(i, j)] + k:C_D[(i, j)] + k + 1]
                    prod = prods.tile([128, D], BF16, tag="prod", name="prod")
                    if i == 3 and j <= 1:
                        nc.gpsimd.tensor_tensor(
                            out=prod[:], in0=h[:], in1=p[:], op=MUL)
                        nc.scalar.activation(prod[:], prod[:], AF.Copy,
                                             accum_out=Dj)
                    else:
                        nc.vector.scalar_tensor_tensor(
                            out=prod[:], in0=h[:], scalar=1.0, in1=p[:],
                            op0=MUL, op1=MUL, accum_out=Dj)

            def emit_recurrence(g, i):
                """Batched [128, GRP] recurrence for group g, layer i -> S."""
                G = gscal[g]
                blk = lambda c: G[:, c:c + GRP]
                if i == 1:
                    return   # S = D0 directly
                if i == 2:
                    nc.vector.tensor_tensor(out=blk(C_T1), in0=blk(C_D[2, 0]),
                                            in1=blk(C_SIG0), op=MUL)
                    nc.vector.tensor_tensor(out=blk(C_T1), in0=blk(C_T1),
                                            in1=blk(C_D[2, 0]), op=ADD)
                    nc.vector.tensor_tensor(out=blk(C_S2), in0=blk(C_T1),
                                            in1=blk(C_D[2, 1]), op=ADD)
                else:
                    nc.vector.tensor_tensor(out=blk(C_T1), in0=blk(C_D[3, 0]),
                                            in1=blk(C_SIG0), op=MUL)
                    nc.vector.tensor_tensor(out=blk(C_T1), in0=blk(C_T1),
                                            in1=blk(C_D[3, 0]), op=ADD)
                    nc.vector.tensor_tensor(out=blk(C_T1), in0=blk(C_T1),
                                            in1=blk(C_D[3, 1]), op=ADD)
                    nc.vector.tensor_tensor(out=blk(C_T2), in0=blk(C_T1),
                                            in1=blk(C_SIG1), op=MUL)
                    nc.vector.tensor_tensor(out=blk(C_T2), in0=blk(C_T2),
                                            in1=blk(C_T1), op=ADD)
                    nc.vector.tensor_tensor(out=blk(C_S3), in0=blk(C_T2),
                                            in1=blk(C_D[3, 2]), op=ADD)

            z_pend = {}

            def emit_gemm(t, i):
                """Transpose + GEMM for (tile t, layer i); z kept pending."""
                h = acts[t][-1]
                tr = tpsum.tile([128, NCH, 128], BF16, tag="tr", name="tr")
                for c in range(NCH):
                    nc.tensor.transpose(
                        tr[:, c, :], h[:, c * 128:(c + 1) * 128], ident[:])
                xT = xTs.tile([128, NCH, 128], BF16, tag="xT", name="xT")
                nc.scalar.activation(xT[:], tr[:], AF.Copy)

                z = zpsum.tile([128, D], F32, tag="z", name="z")
                for c in range(NCH):
                    nc.tensor.matmul(
                        z[:], xT[:, c, :], wt_sb[:, i, c, :],
                        start=(c == 0), stop=False)
                nc.tensor.matmul(
                    z[:], ones_col[:], bias_sb[:, i, :],
                    start=False, stop=True)
                z_pend[t] = z

            def emit_evac(t, i):
                """PSUM -> SBUF evacuation (rank-1 + bias fused) for tile t."""
                g, k = t // GRP, t % GRP
                z = z_pend.pop(t)
                y = y_tiles[t][:, i, :]
                if i == 0:
                    sig0 = gscal[g][:, C_SIG0 + k:C_SIG0 + k + 1]
                    nc.scalar.activation(y, z[:], AF.Copy, accum_out=sig0)
                else:
                    acc = None
                    if i == 1:
                        acc = gscal[g][:, C_SIG1 + k:C_SIG1 + k + 1]
                        S_col = gscal[g][:, C_D[1, 0] + k:C_D[1, 0] + k + 1]
                    elif i == 2:
                        S_col = gscal[g][:, C_S2 + k:C_S2 + k + 1]
                    else:
                        S_col = gscal[g][:, C_S3 + k:C_S3 + k + 1]
                    nc.vector.scalar_tensor_tensor(
                        out=y, in0=rb_sb[:, i, :], scalar=S_col,
                        in1=z[:], op0=MUL, op1=ADD, accum_out=acc)

                if i == NUM_LAYERS - 1:
                    nc.sync.dma_start(out_dram[t, :, :], y_tiles[t][:])
                acts[t].append(y)

            for s in range(ntiles + WAVE * (NUM_LAYERS - 1)):
                for i in range(NUM_LAYERS):
                    t = s - WAVE * i
                    if 0 <= t < ntiles:
                        if i >= 1:
                            emit_dots(t, i)
                            if t % GRP == GRP - 1:
                                emit_recurrence(t // GRP, i)
                        emit_gemm(t, i)
                        if i <= 1:
                            # S is available immediately (or not needed)
                            emit_evac(t, i)
                        elif t % GRP == GRP - 1:
                            # S for the whole group just became available
                            for tt in range(t - GRP + 1, t + 1):
                                emit_evac(tt, i)

    nc.compile()
    return nc


def _host_prep(W, b):
    """W [L,D,D] f32 (torch Linear: y = x @ W.T) -> bf16 moving operand
    WT[l,d,e] = W[l,e,d], bf16 bias, f32 row-sum broadcast tiles."""
    import ml_dtypes
    WT = np.ascontiguousarray(W.transpose(0, 2, 1))
    r = W.sum(axis=2, dtype=np.float64).astype(np.float32)      # [L, D]
    rb = np.ascontiguousarray(
        np.broadcast_to(r[:, None, :], (NUM_LAYERS, 128, D)), dtype=np.float32)
    return (np.asarray(WT, dtype=ml_dtypes.bfloat16),
            np.asarray(b, dtype=ml_dtypes.bfloat16),
            rb)


def run_shards(x, W, b, **spmd_kwargs):
    """Run the SPMD kernel; returns (full_output, BassKernelResults)."""
    import ml_dtypes
    from concourse.bass_utils import run_bass_kernel_spmd

    x_bf = np.asarray(np.asarray(x, np.float32), dtype=ml_dtypes.bfloat16)
    WT, bias, rb = _host_prep(np.asarray(W, np.float32),
                              np.asarray(b, np.float32))

    if "nc" not in _CACHE:
        _CACHE["nc"] = _build_nc()
    nc = _CACHE["nc"]

    in_maps = []
    for c in range(N_CORES):
        shard = x_bf[c * ROWS_PER_CORE:(c + 1) * ROWS_PER_CORE]
        in_maps.append({"x": np.ascontiguousarray(shard), "wt": WT,
                        "bias": bias, "rb": rb})

    res = run_bass_kernel_spmd(nc, in_maps, core_ids=list(range(N_CORES)),
                               **spmd_kwargs)
    out = np.concatenate(
        [np.asarray(r["out"], dtype=np.float32) for r in res.results], axis=0)
    return out, res


def kernel(x, W, b):
    out, _ = run_shards(x, W, b)
    return out


# revision 18
# speedup vs baseline: 1.1762x; 1.1762x over previous
"""Trainium2 Bass kernel for nn_CrossNetwork: 4-layer cross-network.

Reference semantics (per row b of x [B, D], D=512, L=4 layers):
    x_list = [x]
    for i in range(L):
        h = x_list[-1]
        for p in x_list[:-1]:          # sequential dot-product residuals
            s = <h_cur, p>             # scalar per row (h_cur updated each step)
            h_cur = h_cur + s * ones
        y = h_cur @ W[i].T + b[i]
        x_list.append(y)
    out = concat(x_list[1:])           # [B, L*D]

Algebraic restructure (exact): with D_j = <h, p_j> (h unmodified) and
sig_j = rowsum(p_j), the recurrence s'_j = D_j + S_{<j}*sig_j, S = sum s'_j
gives x_fin = h + S.  Then
    y = x_fin @ W_i^T + b = (h @ W_i^T) + S * r_i + b,   r_i = rowsum(W_i)
so the GEMM runs on h = y_{i-1} directly and the cross term is a rank-1
update applied during PSUM evacuation:
    y = (r_bcast * S_col) + z        (z = h @ W_i^T + b via PE accumulation)

Scheduling: engine queues execute in program order, so work is emitted as a
WAVEFRONT: tile t's layer i lands at slot s = t + 4*i; each slot mixes
tiles at different layers, smoothing the layer-dependent dot load.  All 16
tiles' activations stay SBUF-resident.  PE warm-up dummies + front-loaded
x/W0 DMA kicks (spread across engine queues) avoid the cold-clock start.

The per-row scalar recurrence is BATCHED over groups of 4 tiles: dot/sigma
accumulators land in per-group [128, 4] column blocks, and the recurrence
runs as a handful of tiny [128, 4] tensor_tensor ops on DVE instead of
16x as many per-tile column ops.

Engine mapping per (tile, layer):
  PE : 4 transposes of h chunks, 4 GEMM matmuls (N=512, PSUM-accumulate),
       1 bias matmul (K=1, ones x b_row)
  ACT: PSUM->SBUF bf16 copy of transposed chunks; layer-0 evacuation
       (sigma0 via accum_out); reduce half of GPSIMD-multiplied dots
  DVE: most dots as one fused scalar_tensor_tensor (accum_out -> D col);
       rank-1 fused evacuation stt (sigma1 via accum_out); batched
       recurrence [128,4] ops
  GPS: multiply half of two layer-3 dots per tile; x-tile DMA kicks

Everything bf16 (x, W, activations, output -> 13 MiB DMA/core); output is
written bf16, one 512 KB DMA per tile, upcast on host.  Rel err ~7e-3.
Sharding: batch split across 8 NeuronCores (data parallel, SPMD).
"""

import numpy as np

NUM_LAYERS = 4
D = 512
B = 16384
N_CORES = 8
ROWS_PER_CORE = B // N_CORES          # 2048
NTILES = ROWS_PER_CORE // 128         # 16
NCH = D // 128                        # 4 contraction chunks
WAVE = 4                              # slot stagger between layers
GRP = 4                               # tiles per recurrence batch group

_CACHE = {}


def _build_nc(ntiles=NTILES):
    import concourse.tile as tile
    from concourse import bacc, mybir
    from concourse.masks import make_identity

    F32 = mybir.dt.float32
    BF16 = mybir.dt.bfloat16
    AF = mybir.ActivationFunctionType
    MUL = mybir.AluOpType.mult
    ADD = mybir.AluOpType.add

    rows = ntiles * 128
    ngrp = ntiles // GRP

    nc = bacc.Bacc("TRN2", target_bir_lowering=False, debug=False)

    X = nc.dram_tensor("x", [rows, D], BF16, kind="ExternalInput")
    WT = nc.dram_tensor("wt", [NUM_LAYERS, D, D], BF16, kind="ExternalInput")
    BIAS = nc.dram_tensor("bias", [NUM_LAYERS, D], BF16, kind="ExternalInput")
    RB = nc.dram_tensor("rb", [NUM_LAYERS, 128, D], F32, kind="ExternalInput")
    OUT = nc.dram_tensor("out", [rows, NUM_LAYERS * D], BF16,
                         kind="ExternalOutput")

    with tile.TileContext(nc) as tc:
        with (
            tc.tile_pool(name="consts", bufs=1) as consts,
            tc.tile_pool(name="xs", bufs=1) as xs,
            tc.tile_pool(name="ys", bufs=1) as ys,
            tc.tile_pool(name="scals", bufs=1) as scals,
            tc.tile_pool(name="xTs", bufs=8) as xTs,
            tc.tile_pool(name="prods", bufs=4) as prods,
            tc.tile_pool(name="zpsum", bufs=4, space="PSUM") as zpsum,
            tc.tile_pool(name="tpsum", bufs=4, space="PSUM") as tpsum,
        ):
            # ---- identity + PE warm-up (runs while DMAs land) ----
            ones_f32 = consts.tile([1, 128], F32)
            nc.vector.memset(ones_f32[:], 1.0)
            ones_col = consts.tile([1, 128], BF16)
            nc.vector.tensor_copy(ones_col[:], ones_f32[:])
            ident = consts.tile([128, 128], BF16)
            make_identity(nc, ident[:])
            warm = tpsum.tile([128, NCH, 128], BF16, tag="tr", name="warm")
            for _ in range(40):
                nc.tensor.transpose(warm[:, 0, :], ident[:], ident[:])

            # ---- inputs: x tiles + layer-0 weights first, kicks spread ----
            x_dram = X.rearrange("(t p) d -> t p d", p=128)
            out_dram = OUT.rearrange("(t p) d -> t p d", p=128)
            wt_dram = WT.rearrange("l (c p) e -> l c p e", p=128)

            wt_sb = consts.tile([128, NUM_LAYERS, NCH, D], BF16)
            bias_sb = consts.tile([1, NUM_LAYERS, D], BF16)
            rb_sb = consts.tile([128, NUM_LAYERS, D], F32)

            x_tiles, y_tiles = [], []
            for t in range(ntiles):
                x_t = xs.tile([128, D], BF16, tag=f"x{t}", name=f"x{t}")
                x_tiles.append(x_t)
                y_tiles.append(ys.tile([128, NUM_LAYERS, D], BF16,
                                       tag=f"y{t}", name=f"y{t}"))
            gscal = [scals.tile([128, 64], F32, tag=f"g{g}", name=f"g{g}")
                     for g in range(ngrp)]

            # layer-0 critical loads first; x from GPSIMD queue, W from SP,
            # bias/rb from DVE queue so the kick costs parallelize.
            for t in range(4):
                nc.gpsimd.dma_start(x_tiles[t][:], x_dram[t, :, :])
            for c in range(NCH):
                nc.sync.dma_start(wt_sb[:, 0, c, :], wt_dram[0, c, :, :])
            nc.sync.dma_start(bias_sb[0:1, 0, :], BIAS[0:1, :])
            for t in range(4, ntiles):
                nc.gpsimd.dma_start(x_tiles[t][:], x_dram[t, :, :])
            for i in range(1, NUM_LAYERS):
                for c in range(NCH):
                    nc.sync.dma_start(wt_sb[:, i, c, :], wt_dram[i, c, :, :])
                nc.scalar.dma_start(bias_sb[0:1, i, :], BIAS[i:i + 1, :])
                nc.scalar.dma_start(rb_sb[:, i, :], RB[i, :, :])

            acts = [[x_tiles[t]] for t in range(ntiles)]

            # gscal column layout (all [128, GRP] blocks, k = t % GRP):
            C_D = {(1, 0): 0, (2, 0): 4, (2, 1): 8,
                   (3, 0): 12, (3, 1): 16, (3, 2): 20}   # D_j at layer i
            C_SIG0, C_SIG1 = 24, 28
            C_S2, C_S3 = 32, 36
            C_T1, C_T2 = 40, 44

            def emit_dots(t, i):
                """Dot products for (tile t, layer i) -> accumulator cols."""
                g, k = t // GRP, t % GRP
                h = acts[t][-1]
                for j, p in enumerate(acts[t][:-1]):
                    Dj = gscal[g][:, C_D[(i, j)] + k:C_D[(i, j)] + k + 1]
                    prod = prods.tile([128, D], BF16, tag="prod", name="prod")
                    if i == 3 and j <= 1:
                        nc.gpsimd.tensor_tensor(
                            out=prod[:], in0=h[:], in1=p[:], op=MUL)
                        nc.scalar.activation(prod[:], prod[:], AF.Copy,
                                             accum_out=Dj)
                    else:
                        nc.vector.scalar_tensor_tensor(
                            out=prod[:], in0=h[:], scalar=1.0, in1=p[:],
                            op0=MUL, op1=MUL, accum_out=Dj)

            def emit_recurrence(g, i):
                """Batched [128, GRP] recurrence for group g, layer i -> S."""
                G = gscal[g]
                blk = lambda c: G[:, c:c + GRP]
                if i == 1:
                    return   # S = D0 directly
                if i == 2:
                    nc.vector.tensor_tensor(out=blk(C_T1), in0=blk(C_D[2, 0]),
                                            in1=blk(C_SIG0), op=MUL)
                    nc.vector.tensor_tensor(out=blk(C_T1), in0=blk(C_T1),
                                            in1=blk(C_D[2, 0]), op=ADD)
                    nc.vector.tensor_tensor(out=blk(C_S2), in0=blk(C_T1),
                                            in1=blk(C_D[2, 1]), op=ADD)
                else:
                    nc.vector.tensor_tensor(out=blk(C_T1), in0=blk(C_D[3, 0]),
                                            in1=blk(C_SIG0), op=MUL)
                    nc.vector.tensor_tensor(out=blk(C_T1), in0=blk(C_T1),
                                            in1=blk(C_D[3, 0]), op=ADD)
                    nc.vector.tensor_tensor(out=blk(C_T1), in0=blk(C_T1),
                                            in1=blk(C_D[3, 1]), op=ADD)
                    nc.vector.tensor_tensor(out=blk(C_T2), in0=blk(C_T1),
                                            in1=blk(C_SIG1), op=MUL)
                    nc.vector.tensor_tensor(out=blk(C_T2), in0=blk(C_T2),
                                            in1=blk(C_T1), op=ADD)
                    nc.vector.tensor_tensor(out=blk(C_S3), in0=blk(C_T2),
                                            in1=blk(C_D[3, 2]), op=ADD)

            z_pend = {}

            def emit_gemm(t, i):
                """Transpose + GEMM for (tile t, layer i); z kept pending."""
                h = acts[t][-1]
                tr = tpsum.tile([128, NCH, 128], BF16, tag="tr", name="tr")
                for c in range(NCH):
                    nc.tensor.transpose(
                        tr[:, c, :], h[:, c * 128:(c + 1) * 128], ident[:])
                xT = xTs.tile([128, NCH, 128], BF16, tag="xT", name="xT")
                nc.scalar.activation(xT[:], tr[:], AF.Copy)

                z = zpsum.tile([128, D], F32, tag="z", name="z")
                for c in range(NCH):
                    nc.tensor.matmul(
                        z[:], xT[:, c, :], wt_sb[:, i, c, :],
                        start=(c == 0), stop=False)
                nc.tensor.matmul(
                    z[:], ones_col[:], bias_sb[:, i, :],
                    start=False, stop=True)
                z_pend[t] = z

            def emit_evac(t, i):
                """PSUM -> SBUF evacuation (rank-1 + bias fused) for tile t."""
                g, k = t // GRP, t % GRP
                z = z_pend.pop(t)
                y = y_tiles[t][:, i, :]
                if i == 0:
                    sig0 = gscal[g][:, C_SIG0 + k:C_SIG0 + k + 1]
                    nc.scalar.activation(y, z[:], AF.Copy, accum_out=sig0)
                else:
                    acc = None
                    if i == 1:
                        acc = gscal[g][:, C_SIG1 + k:C_SIG1 + k + 1]
                        S_col = gscal[g][:, C_D[1, 0] + k:C_D[1, 0] + k + 1]
                    elif i == 2:
                        S_col = gscal[g][:, C_S2 + k:C_S2 + k + 1]
                    else:
                        S_col = gscal[g][:, C_S3 + k:C_S3 + k + 1]
                    nc.vector.scalar_tensor_tensor(
                        out=y, in0=rb_sb[:, i, :], scalar=S_col,
                        in1=z[:], op0=MUL, op1=ADD, accum_out=acc)

                if i == NUM_LAYERS - 1:
                    nc.sync.dma_start(out_dram[t, :, :], y_tiles[t][:])
                acts[t].append(y)

            for s in range(ntiles + WAVE * (NUM_LAYERS - 1)):
                for i in range(NUM_LAYERS):
                    t = s - WAVE * i
                    if 0 <= t < ntiles:
                        if i >= 1:
                            emit_dots(t, i)
                            if t % GRP == GRP - 1:
                                emit_recurrence(t // GRP, i)
                        emit_gemm(t, i)
                        if i <= 1:
                            # S is available immediately (or not needed)
                            emit_evac(t, i)
                        elif t % GRP == GRP - 1:
                            # S for the whole group just became available
                            for tt in range(t - GRP + 1, t + 1):
                                emit_evac(tt, i)

    nc.compile()
    return nc


def _host_prep(W, b):
    """W [L,D,D] f32 (torch Linear: y = x @ W.T) -> bf16 moving operand
    WT[l,d,e] = W[l,e,d], bf16 bias, f32 row-sum broadcast tiles."""
    import ml_dtypes
    WT = np.ascontiguousarray(W.transpose(0, 2, 1))
    r = W.sum(axis=2, dtype=np.float64).astype(np.float32)      # [L, D]
    rb = np.ascontiguousarray(
        np.broadcast_to(r[:, None, :], (NUM_LAYERS, 128, D)), dtype=np.float32)
    return (np.asarray(WT, dtype=ml_dtypes.bfloat16),
            np.asarray(b, dtype=ml_dtypes.bfloat16),
            rb)


def run_shards(x, W, b, **spmd_kwargs):
    """Run the SPMD kernel; returns (full_output, BassKernelResults)."""
    import ml_dtypes
    from concourse.bass_utils import run_bass_kernel_spmd

    x_bf = np.asarray(np.asarray(x, np.float32), dtype=ml_dtypes.bfloat16)
    WT, bias, rb = _host_prep(np.asarray(W, np.float32),
                              np.asarray(b, np.float32))

    if "nc" not in _CACHE:
        _CACHE["nc"] = _build_nc()
    nc = _CACHE["nc"]

    in_maps = []
    for c in range(N_CORES):
        shard = x_bf[c * ROWS_PER_CORE:(c + 1) * ROWS_PER_CORE]
        in_maps.append({"x": np.ascontiguousarray(shard), "wt": WT,
                        "bias": bias, "rb": rb})

    res = run_bass_kernel_spmd(nc, in_maps, core_ids=list(range(N_CORES)),
                               **spmd_kwargs)
    out = np.concatenate(
        [np.asarray(r["out"], dtype=np.float32) for r in res.results], axis=0)
    return out, res


def kernel(x, W, b):
    out, _ = run_shards(x, W, b)
    return out
